# revision 5
# baseline (speedup 1.0000x reference)
"""DotAttackHead kernel for Trainium2 (8 NeuronCores, data-parallel over batch).

prob = softmax(relu(ufeat @ W.T + b) @ efeat.T / sqrt(256) + mask_bias)
W = g * v / ||v||_F

Sharding: batch 64 -> 8 cores x 8 batches (data-parallel). Params replicated.

Host prep: weight-norm W, transpose+bf16-cast of ufeat/efeat (the PE needs
the contraction dim on partitions, and bf16 halves input DMA), and the mask
folded into efeat: masked columns (n >= num_enemy) are set to -1e30, so
masked logits land at <= -1e28 and exp underflows to exactly 0 — the same 0
the reference's -1e9 bias produces.

Device per batch b (software-pipelined across batches):
  mm1:  projT[e,u] = relu(wT.T @ ufT[b] + bias)   (PE bf16; bias+relu fused
        on DVE as tensor_scalar add/max reading PSUM, bf16 out)
  mm2:  psum[u,n]  = projT.T @ efT[b]             (PE bf16, fp32 PSUM)
  soft: e = Exp(psum/16) with accum_out row-sum for free (ACT), r = 1/s
        (DVE reciprocal), prob = e * r (DVE 4x bf16), bf16 DMA out
        (host upcasts to f32).
No max-subtraction: logits are O(+-6) so exp is safe in fp32, and softmax is
shift-invariant, so this matches the reference.

Mask-width specialization: masked output columns are exactly 0, so the
program is compiled (per num_enemy multiset, NEFF-cached) with a static
per-slot column budget: batches sorted by effective width descending,
rank 8k+c -> (core c, slot k), slot width = slot max rounded up to 128.
Only columns [0, W_k) are computed/stored; the rest of each output row is
zeroed (device writes nothing there; host also zeroes defensively).

v2 scheduling (from NTFF profile of the 84-86us baseline):
- Stores issue on the gpsimd SWDGE ring, loads on the Sync HWDGE ring:
  separate DMA rings round-robin at packet granularity, so a store whose
  data isn't ready can no longer head-of-line-block the next batch's loads
  (single-ring FIFO did exactly that), and SDMA drains both streams.
- uft loads are one DMA per batch (2KB descriptors; the old u-half split
  halved descriptor size for no steady-state benefit). Batch 0 keeps the
  split so mm1 starts after 512KB.
- Load lookahead 3 batches (pin bufs=4) keeps ~4MB queued on the load ring.
- PE chains alternate PSUM banks instruction-by-instruction (mm1: the two
  e-half accumulation chains interleaved; mm2: two u-tiles interleaved), so
  consecutive MATMULs hit different banks and fill/drain pipelines instead
  of serializing at the isolated-MM gap.
"""

from contextlib import ExitStack

import ml_dtypes
import numpy as np

import concourse.bass as bass
import concourse.mybir as mybir
import concourse.tile as tile
from concourse import bacc
from concourse.bass_utils import run_bass_kernel_spmd

N_CORES = 8
B = 64
U = 1024  # units
E = 256   # efeat dim
K = 512   # ufeat dim
N = 1024  # enemies
BPC = B // N_CORES  # batches per core
# processing order over descending-width slot ranks (see run())
SLOT_SIGMA = [6, 0, 1, 2, 3, 5, 7, 4]

F32 = mybir.dt.float32
BF16 = mybir.dt.bfloat16
BF16_NP = ml_dtypes.bfloat16

def _build_bass(bpc: int = BPC, widths: tuple = ()) -> bass.Bass:
    if not widths:
        widths = (N,) * bpc
    assert len(widths) == bpc and all(w % 32 == 0 and 128 <= w <= N for w in widths)
    # Bacc (not raw Bass): its finalize() runs generate_event_semaphores,
    # which splits multi-wait instructions to satisfy TRN2's 1-wait limit.
    nc = bacc.Bacc(None, target_bir_lowering=False)

    ufT = nc.declare_dram_parameter("ufT", [bpc, K, U], BF16, isOutput=False)
    efT = nc.declare_dram_parameter("efT", [bpc, E, N], BF16, isOutput=False)
    wT = nc.declare_dram_parameter("wT", [K, E], BF16, isOutput=False)
    bias = nc.declare_dram_parameter("bias", [E], F32, isOutput=False)
    # bf16 output store halves the dominant DMA stream; host upcasts to f32.
    prob = nc.declare_dram_parameter("prob", [bpc, U, N], BF16, isOutput=True)

    with tile.TileContext(nc) as tc, ExitStack() as ctx:
        singles = ctx.enter_context(tc.tile_pool(name="singles", bufs=1))
        pin = ctx.enter_context(tc.tile_pool(name="pin", bufs=4))
        pproj = ctx.enter_context(tc.tile_pool(name="pproj", bufs=3))
        pprob = ctx.enter_context(tc.tile_pool(name="pprob", bufs=3))
        psmall = ctx.enter_context(tc.tile_pool(name="psmall", bufs=16))
        pps1 = ctx.enter_context(tc.tile_pool(name="pps1", bufs=2, space="PSUM"))
        pps2 = ctx.enter_context(tc.tile_pool(name="pps2", bufs=3, space="PSUM"))

        # ---- resident constants ----
        # wT as 4 k-tiles: wt_sb[p, kt, e] = wT[kt*128+p, e]
        wt_sb = singles.tile([128, 4, E], BF16)
        # scalar (ACT) HWDGE ring: wt/bias stream concurrently with uft0 on
        # the Sync ring instead of ahead of it
        nc.scalar.dma_start(out=wt_sb, in_=wT[:, :].rearrange("(kt p) e -> p kt e", p=128))
        # bias as 2 e-tiles on partitions: b_sb[p, et] = bias[et*128+p]
        b_sb = singles.tile([128, 2], F32)
        nc.scalar.dma_start(out=b_sb, in_=bias[:].rearrange("(et p) -> p et", p=128))

        def emit_loads(bi, slot0=False):
            uft = pin.tile([128, 4, U], BF16, tag="uft")
            W = widths[bi]
            eft = pin.tile([128, 2, W], BF16, tag="eft", name=f"eft{bi}")
            if slot0:
                # First slot ramps the pipeline: u-quarter loads so mm1 can
                # start after 256KB, with eft (small) right after the first
                # quarter so mm2/exp of u-tiles 0-1 unblock ~2us sooner.
                usl = slice(0, 256)
                nc.sync.dma_start(
                    out=uft[:, :, usl],
                    in_=ufT[bi, :, usl].rearrange("(kt p) u -> p kt u", p=128),
                )
                nc.sync.dma_start(
                    out=eft, in_=efT[bi, :, :W].rearrange("(et p) n -> p et n", p=128)
                )
                for q in range(1, 4):
                    usl = slice(q * 256, (q + 1) * 256)
                    nc.sync.dma_start(
                        out=uft[:, :, usl],
                        in_=ufT[bi, :, usl].rearrange("(kt p) u -> p kt u", p=128),
                    )
            else:
                nc.sync.dma_start(
                    out=uft, in_=ufT[bi, :, :].rearrange("(kt p) u -> p kt u", p=128)
                )
                nc.sync.dma_start(
                    out=eft, in_=efT[bi, :, :W].rearrange("(et p) n -> p et n", p=128)
                )
            return uft, eft

        def emit_mm1_pair(uft, projT, uc, quarter=False):
            # Both e-half accumulation chains of one u-chunk, interleaved so
            # consecutive MATMULs target different PSUM banks (fill/drain
            # pipelining); each chain's start/stop accumulation unchanged.
            # quarter=True (slot-0 ramp) runs 256-wide so the first chain
            # starts after a quarter of uft has landed.
            fd = 256 if quarter else 512
            usl = slice(uc * fd, (uc + 1) * fd)
            ps = [
                pps1.tile([128, fd], F32, tag="ps1", name=f"ps1_{uc}_{ej}")
                for ej in range(2)
            ]
            for kj in range(4):
                for ej in range(2):
                    nc.tensor.matmul(
                        ps[ej],
                        lhsT=wt_sb[:, kj, ej * 128 : (ej + 1) * 128],
                        rhs=uft[:, kj, usl],
                        start=(kj == 0),
                        stop=(kj == 3),
                    )
            for ej in range(2):
                # relu(x + b) = max(x + b, 0) fused on DVE; casts to bf16
                nc.vector.tensor_scalar(
                    out=projT[:, ej, usl],
                    in0=ps[ej],
                    scalar1=b_sb[:, ej : ej + 1],
                    scalar2=0.0,
                    op0=mybir.AluOpType.add,
                    op1=mybir.AluOpType.max,
                )

        pair_state = {}

        def emit_softmax_tail(bi, ui, ps2):
            # exp lands directly in the store-gang slot; the normalize is an
            # in-place DVE multiply (no separate et tile, fewer sem edges).
            # Last batch stores in 2-tile gangs so the final drain overlaps
            # the remaining tiles' compute; others gang 4 (fewer stores).
            W = widths[bi]
            gang = 2 if bi == bpc - 1 else 4
            if ui % gang == 0:
                pair_state["tile"] = pprob.tile(
                    [128, gang, W], BF16, tag="prob", name=f"prob{bi}_{ui}"
                )
            prob_t = pair_state["tile"]
            slot = prob_t[:, ui % gang, :]
            s = psmall.tile([128, 1], F32, tag="s")
            nc.scalar.activation(
                out=slot,
                in_=ps2,
                func=mybir.ActivationFunctionType.Exp,
                scale=1.0 / 16.0,
                accum_out=s,
            )
            r = psmall.tile([128, 1], F32, tag="r")
            nc.vector.reciprocal(out=r, in_=s)
            nc.vector.tensor_scalar_mul(out=slot, in0=slot, scalar1=r)
            if ui % gang == gang - 1:
                base = (ui - (gang - 1)) * 128
                # SWDGE ring (gpsimd): stores never block loads on the Sync ring
                nc.gpsimd.dma_start(
                    out=prob[bi, base : base + 128 * gang, :W].rearrange(
                        "(j p) n -> p j n", p=128
                    ),
                    in_=prob_t,
                )

        def emit_softmax_pair(bi, projT, eft, ui):
            # mm2 for u-tiles ui and ui+1 with chains interleaved across two
            # PSUM tiles (bank alternation), then the two softmax tails.
            W = widths[bi]
            nslices = [slice(0, min(512, W))] + ([slice(512, W)] if W > 512 else [])
            ps2 = [
                pps2.tile([128, W], F32, tag="ps2", name=f"ps2_{bi}_{ui + i}")
                for i in range(2)
            ]
            for ej in range(2):
                for nsl in nslices:
                    for i in range(2):
                        uslice = slice((ui + i) * 128, (ui + i + 1) * 128)
                        nc.tensor.matmul(
                            ps2[i][:, nsl],
                            lhsT=projT[:, ej, uslice],
                            rhs=eft[:, ej, nsl],
                            start=(ej == 0),
                            stop=(ej == 1),
                        )
            for i in range(2):
                emit_softmax_tail(bi, ui + i, ps2[i])

        # Software-pipelined emission: mm1 pairs for batch bi+1 are emitted
        # between softmax pairs of batch bi's second half, so the PE never
        # monopolizes a contiguous window on mm1 while ACT's PSUM backlog
        # drains. Loads run 3 batches ahead on the Sync ring.
        tiles = {0: emit_loads(0, slot0=True)}
        projs = {0: pproj.tile([128, 2, U], BF16, tag="projT", name="projT0")}
        for bi in range(1, min(3, bpc)):
            tiles[bi] = emit_loads(bi)
        for uc in range(4):
            emit_mm1_pair(tiles[0][0], projs[0], uc, quarter=True)
        for bi in range(bpc):
            uft, eft = tiles[bi]
            projT = projs[bi]
            if bi + 3 < bpc:
                tiles[bi + 3] = emit_loads(bi + 3)
            emit_softmax_pair(bi, projT, eft, 0)
            emit_softmax_pair(bi, projT, eft, 2)
            if bi + 1 < bpc:
                projs[bi + 1] = pproj.tile(
                    [128, 2, U], BF16, tag="projT", name=f"projT{bi + 1}"
                )
            emit_softmax_pair(bi, projT, eft, 4)
            if bi + 1 < bpc:
                emit_mm1_pair(tiles[bi + 1][0], projs[bi + 1], 0)
            emit_softmax_pair(bi, projT, eft, 6)
            if bi + 1 < bpc:
                emit_mm1_pair(tiles[bi + 1][0], projs[bi + 1], 1)

    # Runs Bacc.compile(): register allocation + event-semaphore splitting.
    nc.finalize()
    return nc


def _prep_inputs(ufeat, efeat, num_enemy, v, g, b):
    """Host-side prep: weight-norm, transpose + bf16 cast, mask bias."""
    ufeat = np.asarray(ufeat, dtype=np.float32)
    efeat = np.asarray(efeat, dtype=np.float32)
    num_enemy = np.asarray(num_enemy).astype(np.int64)
    v = np.asarray(v, dtype=np.float32)
    g = np.float32(np.asarray(g))
    b = np.asarray(b, dtype=np.float32)

    W = (g / np.float32(np.linalg.norm(v))) * v  # [E, K]
    wT = np.ascontiguousarray(W.T).astype(BF16_NP)  # [K, E]

    # [B, K, U] / [B, E, N] bf16 (cast first: halves the transpose traffic)
    ufT = ufeat.astype(BF16_NP).transpose(0, 2, 1)
    efT = np.ascontiguousarray(efeat.astype(BF16_NP).transpose(0, 2, 1))

    # Mask: poison masked efeat columns (n >= num_enemy) with -1e30. Since
    # proj >= 0 (relu) and a proj row is never identically 0 in practice,
    # masked logits land at <= -1e28 and exp underflows to exactly 0 — the
    # same 0 the reference's -1e9 bias produces. num_enemy==0 => all lanes
    # masked => the reference's uniform -1e9 shift cancels in softmax =>
    # leave those batches unpoisoned.
    ne = np.where(num_enemy > 0, num_enemy, N)
    col_masked = np.arange(N)[None, :] >= ne[:, None]  # [B, N]
    efT[np.broadcast_to(col_masked[:, None, :], efT.shape)] = BF16_NP(-1e30)

    return ufT, efT, wT, b


_nc_cache: dict[tuple, bass.Bass] = {}


def run(ufeat, efeat, num_enemy, v, g, b, trace=False):
    ufT, efT, wT, b = _prep_inputs(ufeat, efeat, num_enemy, v, g, b)

    # Masked columns (n >= num_enemy) of the output are exactly 0 and the
    # PJRT path donates zero-initialized output buffers, so the kernel only
    # needs to compute/store columns [0, W) per batch. Sort batches by
    # effective width (descending), assign rank 8k+c to (core c, slot k),
    # and compile the program with a static per-slot width = the slot's max
    # rounded up to 128. Identical widths across cores keeps it SPMD.
    ne = np.asarray(num_enemy).astype(np.int64)
    ne_eff = np.where(ne > 0, ne, N)
    order = np.argsort(-ne_eff, kind="stable")  # descending width ranks
    # Processing order (sigma over the descending-width slots): a narrow
    # slot first (small eft -> exp starts ~8us sooner), the widest while
    # DMA is hot, the narrowest second-to-last (its inefficient 256B-
    # descriptor store drains under the last batch), and a mid slot last
    # (short, efficient final drain).
    sigma = SLOT_SIGMA[:BPC] if BPC == 8 else list(range(BPC))
    slot_ne = ne_eff[order].reshape(BPC, N_CORES)[sigma]
    widths = tuple(
        int(max(128, -(-int(m) // 32) * 32)) for m in slot_ne.max(axis=1)
    )

    key = (BPC, widths)
    if key not in _nc_cache:
        _nc_cache[key] = _build_bass(BPC, widths)
    nc = _nc_cache[key]

    in_maps = []
    perms = []
    for c in range(N_CORES):
        perm = order.reshape(BPC, N_CORES)[sigma][:, c]  # batch per slot
        perms.append(perm)
        in_maps.append({"ufT": ufT[perm], "efT": efT[perm], "wT": wT, "bias": b})

    res = run_bass_kernel_spmd(nc, in_maps, list(range(N_CORES)), trace=trace)
    out = np.empty((B, U, N), dtype=np.float32)
    for c in range(N_CORES):
        o = res.results[c]["prob"].astype(np.float32)
        # Columns [W, N) are masked => exactly 0 by construction. The device
        # never writes them (donated output buffers are zero-filled); zero
        # them here too so correctness never rests on buffer-init behavior.
        for k, w in enumerate(widths):
            o[k, :, w:] = 0.0
        out[perms[c]] = o
    return out, res


def kernel(ufeat, efeat, num_enemy, v, g, b):
    out, _ = run(ufeat, efeat, num_enemy, v, g, b, trace=False)
    return out


# revision 6
# speedup vs baseline: 1.1099x; 1.1099x over previous
"""DotAttackHead kernel for Trainium2 (8 NeuronCores, data-parallel over batch).

prob = softmax(relu(ufeat @ W.T + b) @ efeat.T / sqrt(256) + mask_bias)
W = g * v / ||v||_F

Sharding: batch 64 -> 8 cores x 8 batches (data-parallel). Params replicated.

Host prep: weight-norm W, transpose+bf16-cast of ufeat/efeat (the PE needs
the contraction dim on partitions, and bf16 halves input DMA), and the mask
folded into efeat: masked columns (n >= num_enemy) are set to -1e30, so
masked logits land at <= -1e28 and exp underflows to exactly 0 — the same 0
the reference's -1e9 bias produces.

Device per batch b (software-pipelined across batches):
  mm1:  projT[e,u] = relu(wT.T @ ufT[b] + bias)   (PE bf16; bias+relu fused
        on DVE as tensor_scalar add/max reading PSUM, bf16 out)
  mm2:  psum[u,n]  = projT.T @ efT[b]             (PE bf16, fp32 PSUM)
  soft: e = Exp(psum/16) with accum_out row-sum for free (ACT), r = 1/s
        (DVE reciprocal), prob = e * r (DVE 4x bf16), bf16 DMA out
        (host upcasts to f32).
No max-subtraction: logits are O(+-6) so exp is safe in fp32, and softmax is
shift-invariant, so this matches the reference.

Mask-width specialization: masked output columns are exactly 0, so the
program is compiled (per num_enemy multiset, NEFF-cached) with a static
per-slot column budget: batches sorted by effective width descending,
rank 8k+c -> (core c, slot k), slot width = slot max rounded up to 128.
Only columns [0, W_k) are computed/stored; the rest of each output row is
zeroed (device writes nothing there; host also zeroes defensively).

v2 scheduling (from NTFF profile of the 84-86us baseline):
- Stores issue on the gpsimd SWDGE ring, loads on the Sync HWDGE ring:
  separate DMA rings round-robin at packet granularity, so a store whose
  data isn't ready can no longer head-of-line-block the next batch's loads
  (single-ring FIFO did exactly that), and SDMA drains both streams.
- uft loads are one DMA per batch (2KB descriptors; the old u-half split
  halved descriptor size for no steady-state benefit). Batch 0 keeps the
  split so mm1 starts after 512KB.
- Load lookahead 3 batches (pin bufs=4) keeps ~4MB queued on the load ring.
- PE chains alternate PSUM banks instruction-by-instruction (mm1: the two
  e-half accumulation chains interleaved; mm2: two u-tiles interleaved), so
  consecutive MATMULs hit different banks and fill/drain pipelines instead
  of serializing at the isolated-MM gap.
"""

from contextlib import ExitStack

import ml_dtypes
import numpy as np

import concourse.bass as bass
import concourse.mybir as mybir
import concourse.tile as tile
from concourse import bacc
from concourse.bass_utils import run_bass_kernel_spmd

N_CORES = 8
B = 64
U = 1024  # units
E = 256   # efeat dim
K = 512   # ufeat dim
N = 1024  # enemies
BPC = B // N_CORES  # batches per core
# processing order over descending-width slot ranks (see run())
SLOT_SIGMA = [6, 0, 1, 2, 3, 5, 7, 4]

F32 = mybir.dt.float32
BF16 = mybir.dt.bfloat16
BF16_NP = ml_dtypes.bfloat16

def _build_bass(bpc: int = BPC, widths: tuple = ()) -> bass.Bass:
    if not widths:
        widths = (N,) * bpc
    assert len(widths) == bpc and all(w % 32 == 0 and 128 <= w <= N for w in widths)
    # Bacc (not raw Bass): its finalize() runs generate_event_semaphores,
    # which splits multi-wait instructions to satisfy TRN2's 1-wait limit.
    nc = bacc.Bacc(None, target_bir_lowering=False)

    ufT = nc.declare_dram_parameter("ufT", [bpc, K, U], BF16, isOutput=False)
    efT = nc.declare_dram_parameter("efT", [bpc, E, N], BF16, isOutput=False)
    wT = nc.declare_dram_parameter("wT", [K, E], BF16, isOutput=False)
    bias = nc.declare_dram_parameter("bias", [E], F32, isOutput=False)
    # bf16 output store halves the dominant DMA stream; host upcasts to f32.
    prob = nc.declare_dram_parameter("prob", [bpc, U, N], BF16, isOutput=True)

    with tile.TileContext(nc) as tc, ExitStack() as ctx:
        singles = ctx.enter_context(tc.tile_pool(name="singles", bufs=1))
        pin = ctx.enter_context(tc.tile_pool(name="pin", bufs=4))
        pproj = ctx.enter_context(tc.tile_pool(name="pproj", bufs=3))
        pprob = ctx.enter_context(tc.tile_pool(name="pprob", bufs=3))
        psmall = ctx.enter_context(tc.tile_pool(name="psmall", bufs=16))
        pps1 = ctx.enter_context(tc.tile_pool(name="pps1", bufs=2, space="PSUM"))
        pps2 = ctx.enter_context(tc.tile_pool(name="pps2", bufs=3, space="PSUM"))

        # ---- resident constants ----
        # wT as 4 k-tiles: wt_sb[p, kt, e] = wT[kt*128+p, e]
        wt_sb = singles.tile([128, 4, E], BF16)
        # scalar (ACT) HWDGE ring: wt/bias stream concurrently with uft0 on
        # the Sync ring instead of ahead of it
        nc.scalar.dma_start(out=wt_sb, in_=wT[:, :].rearrange("(kt p) e -> p kt e", p=128))
        # bias as 2 e-tiles on partitions: b_sb[p, et] = bias[et*128+p]
        b_sb = singles.tile([128, 2], F32)
        nc.scalar.dma_start(out=b_sb, in_=bias[:].rearrange("(et p) -> p et", p=128))

        def emit_loads(bi, slot0=False):
            uft = pin.tile([128, 4, U], BF16, tag="uft")
            W = widths[bi]
            eft = pin.tile([128, 2, W], BF16, tag="eft", name=f"eft{bi}")
            if slot0:
                # First slot ramps the pipeline: u-quarter loads so mm1 can
                # start after 256KB, with eft (small) right after the first
                # quarter so mm2/exp of u-tiles 0-1 unblock ~2us sooner.
                usl = slice(0, 256)
                nc.sync.dma_start(
                    out=uft[:, :, usl],
                    in_=ufT[bi, :, usl].rearrange("(kt p) u -> p kt u", p=128),
                )
                nc.sync.dma_start(
                    out=eft, in_=efT[bi, :, :W].rearrange("(et p) n -> p et n", p=128)
                )
                for q in range(1, 4):
                    usl = slice(q * 256, (q + 1) * 256)
                    nc.sync.dma_start(
                        out=uft[:, :, usl],
                        in_=ufT[bi, :, usl].rearrange("(kt p) u -> p kt u", p=128),
                    )
            else:
                nc.sync.dma_start(
                    out=uft, in_=ufT[bi, :, :].rearrange("(kt p) u -> p kt u", p=128)
                )
                nc.sync.dma_start(
                    out=eft, in_=efT[bi, :, :W].rearrange("(et p) n -> p et n", p=128)
                )
            return uft, eft

        def emit_mm1_pair(uft, projT, uc, quarter=False):
            # Both e-half accumulation chains of one u-chunk, interleaved so
            # consecutive MATMULs target different PSUM banks (fill/drain
            # pipelining); each chain's start/stop accumulation unchanged.
            # quarter=True (slot-0 ramp) runs 256-wide so the first chain
            # starts after a quarter of uft has landed.
            fd = 256 if quarter else 512
            usl = slice(uc * fd, (uc + 1) * fd)
            ps = [
                pps1.tile([128, fd], F32, tag="ps1", name=f"ps1_{uc}_{ej}")
                for ej in range(2)
            ]
            for kj in range(4):
                for ej in range(2):
                    nc.tensor.matmul(
                        ps[ej],
                        lhsT=wt_sb[:, kj, ej * 128 : (ej + 1) * 128],
                        rhs=uft[:, kj, usl],
                        start=(kj == 0),
                        stop=(kj == 3),
                    )
            for ej in range(2):
                # relu(x + b) = max(x + b, 0) fused on DVE; casts to bf16
                nc.vector.tensor_scalar(
                    out=projT[:, ej, usl],
                    in0=ps[ej],
                    scalar1=b_sb[:, ej : ej + 1],
                    scalar2=0.0,
                    op0=mybir.AluOpType.add,
                    op1=mybir.AluOpType.max,
                )

        pair_state = {}

        def emit_softmax_tail(bi, ui, ps2):
            # exp lands directly in the store-gang slot; the normalize is an
            # in-place DVE multiply (no separate et tile, fewer sem edges).
            # Last batch stores in 2-tile gangs so the final drain overlaps
            # the remaining tiles' compute; others gang 4 (fewer stores).
            W = widths[bi]
            gang = 2 if bi == bpc - 1 else 4
            if ui % gang == 0:
                pair_state["tile"] = pprob.tile(
                    [128, gang, W], BF16, tag="prob", name=f"prob{bi}_{ui}"
                )
            prob_t = pair_state["tile"]
            slot = prob_t[:, ui % gang, :]
            s = psmall.tile([128, 1], F32, tag="s")
            nc.scalar.activation(
                out=slot,
                in_=ps2,
                func=mybir.ActivationFunctionType.Exp,
                scale=1.0 / 16.0,
                accum_out=s,
            )
            r = psmall.tile([128, 1], F32, tag="r")
            nc.vector.reciprocal(out=r, in_=s)
            nc.vector.tensor_scalar_mul(out=slot, in0=slot, scalar1=r)
            if ui % gang == gang - 1:
                base = (ui - (gang - 1)) * 128
                # SWDGE ring (gpsimd): stores never block loads on the Sync ring
                nc.gpsimd.dma_start(
                    out=prob[bi, base : base + 128 * gang, :W].rearrange(
                        "(j p) n -> p j n", p=128
                    ),
                    in_=prob_t,
                )

        def emit_softmax_pair(bi, projT, eft, ui):
            # mm2 for u-tiles ui and ui+1 with chains interleaved across two
            # PSUM tiles (bank alternation), then the two softmax tails.
            W = widths[bi]
            nslices = [slice(0, min(512, W))] + ([slice(512, W)] if W > 512 else [])
            ps2 = [
                pps2.tile([128, W], F32, tag="ps2", name=f"ps2_{bi}_{ui + i}")
                for i in range(2)
            ]
            for ej in range(2):
                for nsl in nslices:
                    for i in range(2):
                        uslice = slice((ui + i) * 128, (ui + i + 1) * 128)
                        nc.tensor.matmul(
                            ps2[i][:, nsl],
                            lhsT=projT[:, ej, uslice],
                            rhs=eft[:, ej, nsl],
                            start=(ej == 0),
                            stop=(ej == 1),
                        )
            for i in range(2):
                emit_softmax_tail(bi, ui + i, ps2[i])

        # Software-pipelined emission: mm1 pairs for batch bi+1 are emitted
        # between softmax pairs of batch bi's second half, so the PE never
        # monopolizes a contiguous window on mm1 while ACT's PSUM backlog
        # drains. Loads run 3 batches ahead on the Sync ring.
        tiles = {0: emit_loads(0, slot0=True)}
        projs = {0: pproj.tile([128, 2, U], BF16, tag="projT", name="projT0")}
        for bi in range(1, min(3, bpc)):
            tiles[bi] = emit_loads(bi)
        # slot-0 ramp: each softmax pair directly follows the mm1 quarter it
        # needs, so the first ACTIVATE unblocks ~5us sooner than emitting all
        # of mm1 up front; batch 1's mm1 halves ride along the tail pairs.
        emit_mm1_pair(tiles[0][0], projs[0], 0, quarter=True)
        for bi in range(bpc):
            uft, eft = tiles[bi]
            projT = projs[bi]
            if bi + 3 < bpc:
                tiles[bi + 3] = emit_loads(bi + 3)
            if bi == 0:
                emit_softmax_pair(bi, projT, eft, 0)
                emit_mm1_pair(uft, projT, 1, quarter=True)
                emit_softmax_pair(bi, projT, eft, 2)
                emit_mm1_pair(uft, projT, 2, quarter=True)
                if bpc > 1:
                    projs[1] = pproj.tile(
                        [128, 2, U], BF16, tag="projT", name="projT1"
                    )
                emit_softmax_pair(bi, projT, eft, 4)
                emit_mm1_pair(uft, projT, 3, quarter=True)
                if bpc > 1:
                    emit_mm1_pair(tiles[1][0], projs[1], 0)
                emit_softmax_pair(bi, projT, eft, 6)
                if bpc > 1:
                    emit_mm1_pair(tiles[1][0], projs[1], 1)
                continue
            emit_softmax_pair(bi, projT, eft, 0)
            emit_softmax_pair(bi, projT, eft, 2)
            if bi + 1 < bpc:
                projs[bi + 1] = pproj.tile(
                    [128, 2, U], BF16, tag="projT", name=f"projT{bi + 1}"
                )
            emit_softmax_pair(bi, projT, eft, 4)
            if bi + 1 < bpc:
                emit_mm1_pair(tiles[bi + 1][0], projs[bi + 1], 0)
            emit_softmax_pair(bi, projT, eft, 6)
            if bi + 1 < bpc:
                emit_mm1_pair(tiles[bi + 1][0], projs[bi + 1], 1)

    # Runs Bacc.compile(): register allocation + event-semaphore splitting.
    nc.finalize()
    return nc


def _prep_inputs(ufeat, efeat, num_enemy, v, g, b):
    """Host-side prep: weight-norm, transpose + bf16 cast, mask bias."""
    ufeat = np.asarray(ufeat, dtype=np.float32)
    efeat = np.asarray(efeat, dtype=np.float32)
    num_enemy = np.asarray(num_enemy).astype(np.int64)
    v = np.asarray(v, dtype=np.float32)
    g = np.float32(np.asarray(g))
    b = np.asarray(b, dtype=np.float32)

    W = (g / np.float32(np.linalg.norm(v))) * v  # [E, K]
    wT = np.ascontiguousarray(W.T).astype(BF16_NP)  # [K, E]

    # [B, K, U] / [B, E, N] bf16 (cast first: halves the transpose traffic)
    ufT = ufeat.astype(BF16_NP).transpose(0, 2, 1)
    efT = np.ascontiguousarray(efeat.astype(BF16_NP).transpose(0, 2, 1))

    # Mask: poison masked efeat columns (n >= num_enemy) with -1e30. Since
    # proj >= 0 (relu) and a proj row is never identically 0 in practice,
    # masked logits land at <= -1e28 and exp underflows to exactly 0 — the
    # same 0 the reference's -1e9 bias produces. num_enemy==0 => all lanes
    # masked => the reference's uniform -1e9 shift cancels in softmax =>
    # leave those batches unpoisoned.
    ne = np.where(num_enemy > 0, num_enemy, N)
    col_masked = np.arange(N)[None, :] >= ne[:, None]  # [B, N]
    efT[np.broadcast_to(col_masked[:, None, :], efT.shape)] = BF16_NP(-1e30)

    return ufT, efT, wT, b


_nc_cache: dict[tuple, bass.Bass] = {}


def run(ufeat, efeat, num_enemy, v, g, b, trace=False):
    ufT, efT, wT, b = _prep_inputs(ufeat, efeat, num_enemy, v, g, b)

    # Masked columns (n >= num_enemy) of the output are exactly 0 and the
    # PJRT path donates zero-initialized output buffers, so the kernel only
    # needs to compute/store columns [0, W) per batch. Sort batches by
    # effective width (descending), assign rank 8k+c to (core c, slot k),
    # and compile the program with a static per-slot width = the slot's max
    # rounded up to 128. Identical widths across cores keeps it SPMD.
    ne = np.asarray(num_enemy).astype(np.int64)
    ne_eff = np.where(ne > 0, ne, N)
    order = np.argsort(-ne_eff, kind="stable")  # descending width ranks
    # Processing order (sigma over the descending-width slots): a narrow
    # slot first (small eft -> exp starts ~8us sooner), the widest while
    # DMA is hot, the narrowest second-to-last (its inefficient 256B-
    # descriptor store drains under the last batch), and a mid slot last
    # (short, efficient final drain).
    sigma = SLOT_SIGMA[:BPC] if BPC == 8 else list(range(BPC))
    slot_ne = ne_eff[order].reshape(BPC, N_CORES)[sigma]
    widths = tuple(
        int(max(128, -(-int(m) // 32) * 32)) for m in slot_ne.max(axis=1)
    )

    key = (BPC, widths)
    if key not in _nc_cache:
        _nc_cache[key] = _build_bass(BPC, widths)
    nc = _nc_cache[key]

    in_maps = []
    perms = []
    for c in range(N_CORES):
        perm = order.reshape(BPC, N_CORES)[sigma][:, c]  # batch per slot
        perms.append(perm)
        in_maps.append({"ufT": ufT[perm], "efT": efT[perm], "wT": wT, "bias": b})

    res = run_bass_kernel_spmd(nc, in_maps, list(range(N_CORES)), trace=trace)
    out = np.empty((B, U, N), dtype=np.float32)
    for c in range(N_CORES):
        o = res.results[c]["prob"].astype(np.float32)
        # Columns [W, N) are masked => exactly 0 by construction. The device
        # never writes them (donated output buffers are zero-filled); zero
        # them here too so correctness never rests on buffer-init behavior.
        for k, w in enumerate(widths):
            o[k, :, w:] = 0.0
        out[perms[c]] = o
    return out, res


def kernel(ufeat, efeat, num_enemy, v, g, b):
    out, _ = run(ufeat, efeat, num_enemy, v, g, b, trace=False)
    return out


# revision 7
# speedup vs baseline: 1.1222x; 1.0110x over previous
"""DotAttackHead kernel for Trainium2 (8 NeuronCores, data-parallel over batch).

prob = softmax(relu(ufeat @ W.T + b) @ efeat.T / sqrt(256) + mask_bias)
W = g * v / ||v||_F

Sharding: batch 64 -> 8 cores x 8 batches (data-parallel). Params replicated.

Host prep: weight-norm W, transpose+bf16-cast of ufeat/efeat (the PE needs
the contraction dim on partitions, and bf16 halves input DMA), and the mask
folded into efeat: masked columns (n >= num_enemy) are set to -1e30, so
masked logits land at <= -1e28 and exp underflows to exactly 0 — the same 0
the reference's -1e9 bias produces.

Device per batch b (software-pipelined across batches):
  mm1:  projT[e,u] = relu(wT.T @ ufT[b] + bias)   (PE bf16; bias+relu fused
        on DVE as tensor_scalar add/max reading PSUM, bf16 out)
  mm2:  psum[u,n]  = projT.T @ efT[b]             (PE bf16, fp32 PSUM)
  soft: e = Exp(psum/16) with accum_out row-sum for free (ACT), r = 1/s
        (DVE reciprocal), prob = e * r (DVE 4x bf16), bf16 DMA out
        (host upcasts to f32).
No max-subtraction: logits are O(+-6) so exp is safe in fp32, and softmax is
shift-invariant, so this matches the reference.

Mask-width specialization: masked output columns are exactly 0, so the
program is compiled (per num_enemy multiset, NEFF-cached) with a static
per-slot column budget: batches sorted by effective width descending,
rank 8k+c -> (core c, slot k), slot width = slot max rounded up to 128.
Only columns [0, W_k) are computed/stored; the rest of each output row is
zeroed (device writes nothing there; host also zeroes defensively).

v2 scheduling (from NTFF profile of the 84-86us baseline):
- Stores issue on the gpsimd SWDGE ring, loads on the Sync HWDGE ring:
  separate DMA rings round-robin at packet granularity, so a store whose
  data isn't ready can no longer head-of-line-block the next batch's loads
  (single-ring FIFO did exactly that), and SDMA drains both streams.
- uft loads are one DMA per batch (2KB descriptors; the old u-half split
  halved descriptor size for no steady-state benefit). Batch 0 keeps the
  split so mm1 starts after 512KB.
- Load lookahead 3 batches (pin bufs=4) keeps ~4MB queued on the load ring.
- PE chains alternate PSUM banks instruction-by-instruction (mm1: the two
  e-half accumulation chains interleaved; mm2: two u-tiles interleaved), so
  consecutive MATMULs hit different banks and fill/drain pipelines instead
  of serializing at the isolated-MM gap.
"""

from contextlib import ExitStack

import ml_dtypes
import numpy as np

import concourse.bass as bass
import concourse.mybir as mybir
import concourse.tile as tile
from concourse import bacc
from concourse.bass_utils import run_bass_kernel_spmd

N_CORES = 8
B = 64
U = 1024  # units
E = 256   # efeat dim
K = 512   # ufeat dim
N = 1024  # enemies
BPC = B // N_CORES  # batches per core
# processing order over descending-width slot ranks (see run())
SLOT_SIGMA = [6, 0, 1, 2, 3, 4, 5, 7]

F32 = mybir.dt.float32
BF16 = mybir.dt.bfloat16
BF16_NP = ml_dtypes.bfloat16

def _build_bass(bpc: int = BPC, widths: tuple = ()) -> bass.Bass:
    if not widths:
        widths = (N,) * bpc
    assert len(widths) == bpc and all(w % 32 == 0 and 128 <= w <= N for w in widths)
    # Bacc (not raw Bass): its finalize() runs generate_event_semaphores,
    # which splits multi-wait instructions to satisfy TRN2's 1-wait limit.
    nc = bacc.Bacc(None, target_bir_lowering=False)

    ufT = nc.declare_dram_parameter("ufT", [bpc, K, U], BF16, isOutput=False)
    efT = nc.declare_dram_parameter("efT", [bpc, E, N], BF16, isOutput=False)
    wT = nc.declare_dram_parameter("wT", [K, E], BF16, isOutput=False)
    bias = nc.declare_dram_parameter("bias", [E], F32, isOutput=False)
    # bf16 output store halves the dominant DMA stream; host upcasts to f32.
    prob = nc.declare_dram_parameter("prob", [bpc, U, N], BF16, isOutput=True)

    with tile.TileContext(nc) as tc, ExitStack() as ctx:
        singles = ctx.enter_context(tc.tile_pool(name="singles", bufs=1))
        pin = ctx.enter_context(tc.tile_pool(name="pin", bufs=4))
        pproj = ctx.enter_context(tc.tile_pool(name="pproj", bufs=3))
        pprob = ctx.enter_context(tc.tile_pool(name="pprob", bufs=3))
        psmall = ctx.enter_context(tc.tile_pool(name="psmall", bufs=16))
        pps1 = ctx.enter_context(tc.tile_pool(name="pps1", bufs=2, space="PSUM"))
        pps2 = ctx.enter_context(tc.tile_pool(name="pps2", bufs=3, space="PSUM"))

        # ACT exp-table prefetch: the first Exp triggers a ~1.3us
        # ACT_TABLE_LOAD; run a dummy 1-element exp during the load phase so
        # the first real ACTIVATE doesn't pay it on the critical path.
        warm = singles.tile([128, 1], F32, name="warm")
        nc.gpsimd.memset(warm, 0.0)
        warm_out = singles.tile([128, 1], F32, name="warm_out")
        nc.scalar.activation(
            out=warm_out, in_=warm, func=mybir.ActivationFunctionType.Exp
        )

        # ---- resident constants ----
        # wT as 4 k-tiles: wt_sb[p, kt, e] = wT[kt*128+p, e]
        wt_sb = singles.tile([128, 4, E], BF16)
        # scalar (ACT) HWDGE ring: wt/bias stream concurrently with uft0 on
        # the Sync ring instead of ahead of it
        nc.scalar.dma_start(out=wt_sb, in_=wT[:, :].rearrange("(kt p) e -> p kt e", p=128))
        # bias as 2 e-tiles on partitions: b_sb[p, et] = bias[et*128+p]
        b_sb = singles.tile([128, 2], F32)
        nc.scalar.dma_start(out=b_sb, in_=bias[:].rearrange("(et p) -> p et", p=128))

        def emit_loads(bi, slot0=False):
            uft = pin.tile([128, 4, U], BF16, tag="uft")
            W = widths[bi]
            eft = pin.tile([128, 2, W], BF16, tag="eft", name=f"eft{bi}")
            if slot0:
                # First slot ramps the pipeline: u-quarter loads so mm1 can
                # start after 256KB, with eft (small) right after the first
                # quarter so mm2/exp of u-tiles 0-1 unblock ~2us sooner.
                usl = slice(0, 256)
                nc.sync.dma_start(
                    out=uft[:, :, usl],
                    in_=ufT[bi, :, usl].rearrange("(kt p) u -> p kt u", p=128),
                )
                nc.sync.dma_start(
                    out=eft, in_=efT[bi, :, :W].rearrange("(et p) n -> p et n", p=128)
                )
                for q in range(1, 4):
                    usl = slice(q * 256, (q + 1) * 256)
                    nc.sync.dma_start(
                        out=uft[:, :, usl],
                        in_=ufT[bi, :, usl].rearrange("(kt p) u -> p kt u", p=128),
                    )
            else:
                nc.sync.dma_start(
                    out=uft, in_=ufT[bi, :, :].rearrange("(kt p) u -> p kt u", p=128)
                )
                nc.sync.dma_start(
                    out=eft, in_=efT[bi, :, :W].rearrange("(et p) n -> p et n", p=128)
                )
            return uft, eft

        def emit_mm1_pair(uft, projT, uc, quarter=False):
            # Both e-half accumulation chains of one u-chunk, interleaved so
            # consecutive MATMULs target different PSUM banks (fill/drain
            # pipelining); each chain's start/stop accumulation unchanged.
            # quarter=True (slot-0 ramp) runs 256-wide so the first chain
            # starts after a quarter of uft has landed.
            fd = 256 if quarter else 512
            usl = slice(uc * fd, (uc + 1) * fd)
            ps = [
                pps1.tile([128, fd], F32, tag="ps1", name=f"ps1_{uc}_{ej}")
                for ej in range(2)
            ]
            for kj in range(4):
                for ej in range(2):
                    nc.tensor.matmul(
                        ps[ej],
                        lhsT=wt_sb[:, kj, ej * 128 : (ej + 1) * 128],
                        rhs=uft[:, kj, usl],
                        start=(kj == 0),
                        stop=(kj == 3),
                    )
            for ej in range(2):
                # relu(x + b) = max(x + b, 0) fused on DVE; casts to bf16
                nc.vector.tensor_scalar(
                    out=projT[:, ej, usl],
                    in0=ps[ej],
                    scalar1=b_sb[:, ej : ej + 1],
                    scalar2=0.0,
                    op0=mybir.AluOpType.add,
                    op1=mybir.AluOpType.max,
                )

        pair_state = {}

        def emit_softmax_tail(bi, ui, ps2):
            # exp lands directly in the store-gang slot; the normalize is an
            # in-place DVE multiply (no separate et tile, fewer sem edges).
            # Last batch stores in 2-tile gangs so the final drain overlaps
            # the remaining tiles' compute; others gang 4 (fewer stores).
            W = widths[bi]
            gang = 2 if bi == bpc - 1 else 4
            if ui % gang == 0:
                pair_state["tile"] = pprob.tile(
                    [128, gang, W], BF16, tag="prob", name=f"prob{bi}_{ui}"
                )
            prob_t = pair_state["tile"]
            slot = prob_t[:, ui % gang, :]
            s = psmall.tile([128, 1], F32, tag="s")
            nc.scalar.activation(
                out=slot,
                in_=ps2,
                func=mybir.ActivationFunctionType.Exp,
                scale=1.0 / 16.0,
                accum_out=s,
            )
            r = psmall.tile([128, 1], F32, tag="r")
            nc.vector.reciprocal(out=r, in_=s)
            nc.vector.tensor_scalar_mul(out=slot, in0=slot, scalar1=r)
            if ui % gang == gang - 1:
                base = (ui - (gang - 1)) * 128
                # SWDGE ring (gpsimd): stores never block loads on the Sync ring
                nc.gpsimd.dma_start(
                    out=prob[bi, base : base + 128 * gang, :W].rearrange(
                        "(j p) n -> p j n", p=128
                    ),
                    in_=prob_t,
                )

        def emit_softmax_pair(bi, projT, eft, ui):
            # mm2 for u-tiles ui and ui+1 with chains interleaved across two
            # PSUM tiles (bank alternation), then the two softmax tails.
            W = widths[bi]
            nslices = [slice(0, min(512, W))] + ([slice(512, W)] if W > 512 else [])
            ps2 = [
                pps2.tile([128, W], F32, tag="ps2", name=f"ps2_{bi}_{ui + i}")
                for i in range(2)
            ]
            for ej in range(2):
                for nsl in nslices:
                    for i in range(2):
                        uslice = slice((ui + i) * 128, (ui + i + 1) * 128)
                        nc.tensor.matmul(
                            ps2[i][:, nsl],
                            lhsT=projT[:, ej, uslice],
                            rhs=eft[:, ej, nsl],
                            start=(ej == 0),
                            stop=(ej == 1),
                        )
            for i in range(2):
                emit_softmax_tail(bi, ui + i, ps2[i])

        # Software-pipelined emission: mm1 pairs for batch bi+1 are emitted
        # between softmax pairs of batch bi's second half, so the PE never
        # monopolizes a contiguous window on mm1 while ACT's PSUM backlog
        # drains. Loads run 3 batches ahead on the Sync ring.
        tiles = {0: emit_loads(0, slot0=True)}
        projs = {0: pproj.tile([128, 2, U], BF16, tag="projT", name="projT0")}
        for bi in range(1, min(3, bpc)):
            tiles[bi] = emit_loads(bi)
        # slot-0 ramp: each softmax pair directly follows the mm1 quarter it
        # needs, so the first ACTIVATE unblocks ~5us sooner than emitting all
        # of mm1 up front; batch 1's mm1 halves ride along the tail pairs.
        emit_mm1_pair(tiles[0][0], projs[0], 0, quarter=True)
        for bi in range(bpc):
            uft, eft = tiles[bi]
            projT = projs[bi]
            if bi + 3 < bpc:
                tiles[bi + 3] = emit_loads(bi + 3)
            if bi == 0:
                emit_softmax_pair(bi, projT, eft, 0)
                emit_mm1_pair(uft, projT, 1, quarter=True)
                emit_softmax_pair(bi, projT, eft, 2)
                emit_mm1_pair(uft, projT, 2, quarter=True)
                if bpc > 1:
                    projs[1] = pproj.tile(
                        [128, 2, U], BF16, tag="projT", name="projT1"
                    )
                emit_softmax_pair(bi, projT, eft, 4)
                emit_mm1_pair(uft, projT, 3, quarter=True)
                if bpc > 1:
                    emit_mm1_pair(tiles[1][0], projs[1], 0)
                emit_softmax_pair(bi, projT, eft, 6)
                if bpc > 1:
                    emit_mm1_pair(tiles[1][0], projs[1], 1)
                continue
            emit_softmax_pair(bi, projT, eft, 0)
            if bi + 1 < bpc:
                projs[bi + 1] = pproj.tile(
                    [128, 2, U], BF16, tag="projT", name=f"projT{bi + 1}"
                )
            emit_softmax_pair(bi, projT, eft, 2)
            if bi + 1 < bpc:
                emit_mm1_pair(tiles[bi + 1][0], projs[bi + 1], 0)
            emit_softmax_pair(bi, projT, eft, 4)
            if bi + 1 < bpc:
                emit_mm1_pair(tiles[bi + 1][0], projs[bi + 1], 1)
            emit_softmax_pair(bi, projT, eft, 6)

    # Runs Bacc.compile(): register allocation + event-semaphore splitting.
    nc.finalize()
    return nc


def _prep_inputs(ufeat, efeat, num_enemy, v, g, b):
    """Host-side prep: weight-norm, transpose + bf16 cast, mask bias."""
    ufeat = np.asarray(ufeat, dtype=np.float32)
    efeat = np.asarray(efeat, dtype=np.float32)
    num_enemy = np.asarray(num_enemy).astype(np.int64)
    v = np.asarray(v, dtype=np.float32)
    g = np.float32(np.asarray(g))
    b = np.asarray(b, dtype=np.float32)

    W = (g / np.float32(np.linalg.norm(v))) * v  # [E, K]
    wT = np.ascontiguousarray(W.T).astype(BF16_NP)  # [K, E]

    # [B, K, U] / [B, E, N] bf16 (cast first: halves the transpose traffic)
    ufT = ufeat.astype(BF16_NP).transpose(0, 2, 1)
    efT = np.ascontiguousarray(efeat.astype(BF16_NP).transpose(0, 2, 1))

    # Mask: poison masked efeat columns (n >= num_enemy) with -1e30. Since
    # proj >= 0 (relu) and a proj row is never identically 0 in practice,
    # masked logits land at <= -1e28 and exp underflows to exactly 0 — the
    # same 0 the reference's -1e9 bias produces. num_enemy==0 => all lanes
    # masked => the reference's uniform -1e9 shift cancels in softmax =>
    # leave those batches unpoisoned.
    ne = np.where(num_enemy > 0, num_enemy, N)
    col_masked = np.arange(N)[None, :] >= ne[:, None]  # [B, N]
    efT[np.broadcast_to(col_masked[:, None, :], efT.shape)] = BF16_NP(-1e30)

    return ufT, efT, wT, b


_nc_cache: dict[tuple, bass.Bass] = {}


def run(ufeat, efeat, num_enemy, v, g, b, trace=False):
    ufT, efT, wT, b = _prep_inputs(ufeat, efeat, num_enemy, v, g, b)

    # Masked columns (n >= num_enemy) of the output are exactly 0 and the
    # PJRT path donates zero-initialized output buffers, so the kernel only
    # needs to compute/store columns [0, W) per batch. Sort batches by
    # effective width (descending), assign rank 8k+c to (core c, slot k),
    # and compile the program with a static per-slot width = the slot's max
    # rounded up to 128. Identical widths across cores keeps it SPMD.
    ne = np.asarray(num_enemy).astype(np.int64)
    ne_eff = np.where(ne > 0, ne, N)
    order = np.argsort(-ne_eff, kind="stable")  # descending width ranks
    # Processing order (sigma over the descending-width slots): a narrow
    # slot first (small eft -> exp starts ~8us sooner), the widest while
    # DMA is hot, the narrowest second-to-last (its inefficient 256B-
    # descriptor store drains under the last batch), and a mid slot last
    # (short, efficient final drain).
    sigma = SLOT_SIGMA[:BPC] if BPC == 8 else list(range(BPC))
    slot_ne = ne_eff[order].reshape(BPC, N_CORES)[sigma]
    widths = tuple(
        int(max(128, -(-int(m) // 32) * 32)) for m in slot_ne.max(axis=1)
    )

    key = (BPC, widths)
    if key not in _nc_cache:
        _nc_cache[key] = _build_bass(BPC, widths)
    nc = _nc_cache[key]

    in_maps = []
    perms = []
    for c in range(N_CORES):
        perm = order.reshape(BPC, N_CORES)[sigma][:, c]  # batch per slot
        perms.append(perm)
        in_maps.append({"ufT": ufT[perm], "efT": efT[perm], "wT": wT, "bias": b})

    res = run_bass_kernel_spmd(nc, in_maps, list(range(N_CORES)), trace=trace)
    out = np.empty((B, U, N), dtype=np.float32)
    for c in range(N_CORES):
        o = res.results[c]["prob"].astype(np.float32)
        # Columns [W, N) are masked => exactly 0 by construction. The device
        # never writes them (donated output buffers are zero-filled); zero
        # them here too so correctness never rests on buffer-init behavior.
        for k, w in enumerate(widths):
            o[k, :, w:] = 0.0
        out[perms[c]] = o
    return out, res


def kernel(ufeat, efeat, num_enemy, v, g, b):
    out, _ = run(ufeat, efeat, num_enemy, v, g, b, trace=False)
    return out


# revision 8
# speedup vs baseline: 1.1705x; 1.0431x over previous
"""DotAttackHead kernel for Trainium2 (8 NeuronCores, data-parallel over batch).

prob = softmax(relu(ufeat @ W.T + b) @ efeat.T / sqrt(256) + mask_bias)
W = g * v / ||v||_F

Sharding: batch 64 -> 8 cores x 8 batches (data-parallel). Params replicated.

Host prep: weight-norm W, transpose+bf16-cast of ufeat/efeat (the PE needs
the contraction dim on partitions, and bf16 halves input DMA), and the mask
folded into efeat: masked columns (n >= num_enemy) are set to -1e30, so
masked logits land at <= -1e28 and exp underflows to exactly 0 — the same 0
the reference's -1e9 bias produces.

Device per batch b (software-pipelined across batches):
  mm1:  projT[e,u] = relu(wT.T @ ufT[b] + bias)   (PE bf16; bias+relu fused
        on DVE as tensor_scalar add/max reading PSUM, bf16 out)
  mm2:  psum[u,n]  = projT.T @ efT[b]             (PE bf16, fp32 PSUM)
  soft: e = Exp(psum/16) with accum_out row-sum for free (ACT), r = 1/s
        (DVE reciprocal), prob = e * r (DVE 4x bf16), bf16 DMA out
        (host upcasts to f32).
No max-subtraction: logits are O(+-6) so exp is safe in fp32, and softmax is
shift-invariant, so this matches the reference.

Mask-width specialization: masked output columns are exactly 0, so the
program is compiled (per num_enemy multiset, NEFF-cached) with a static
per-slot column budget: batches sorted by effective width descending,
rank 8k+c -> (core c, slot k), slot width = slot max rounded up to 128.
Only columns [0, W_k) are computed/stored; the rest of each output row is
zeroed (device writes nothing there; host also zeroes defensively).

v2 scheduling (from NTFF profile of the 84-86us baseline):
- Stores issue on the gpsimd SWDGE ring, loads on the Sync HWDGE ring:
  separate DMA rings round-robin at packet granularity, so a store whose
  data isn't ready can no longer head-of-line-block the next batch's loads
  (single-ring FIFO did exactly that), and SDMA drains both streams.
- uft loads are one DMA per batch (2KB descriptors; the old u-half split
  halved descriptor size for no steady-state benefit). Batch 0 keeps the
  split so mm1 starts after 512KB.
- Load lookahead 3 batches (pin bufs=4) keeps ~4MB queued on the load ring.
- PE chains alternate PSUM banks instruction-by-instruction (mm1: the two
  e-half accumulation chains interleaved; mm2: two u-tiles interleaved), so
  consecutive MATMULs hit different banks and fill/drain pipelines instead
  of serializing at the isolated-MM gap.
"""

from contextlib import ExitStack

import ml_dtypes
import numpy as np

import concourse.bass as bass
import concourse.mybir as mybir
import concourse.tile as tile
from concourse import bacc
from concourse.bass_utils import run_bass_kernel_spmd

N_CORES = 8
B = 64
U = 1024  # units
E = 256   # efeat dim
K = 512   # ufeat dim
N = 1024  # enemies
BPC = B // N_CORES  # batches per core
# processing order over descending-width slot ranks (see run())
SLOT_SIGMA = [6, 0, 1, 2, 3, 4, 5, 7]

F32 = mybir.dt.float32
BF16 = mybir.dt.bfloat16
BF16_NP = ml_dtypes.bfloat16

def _build_bass(bpc: int = BPC, widths: tuple = ()) -> bass.Bass:
    if not widths:
        widths = (N,) * bpc
    assert len(widths) == bpc and all(w % 32 == 0 and 128 <= w <= N for w in widths)
    # Bacc (not raw Bass): its finalize() runs generate_event_semaphores,
    # which splits multi-wait instructions to satisfy TRN2's 1-wait limit.
    nc = bacc.Bacc(None, target_bir_lowering=False)

    ufT = nc.declare_dram_parameter("ufT", [bpc, K, U], BF16, isOutput=False)
    efT = nc.declare_dram_parameter("efT", [bpc, E, N], BF16, isOutput=False)
    wT = nc.declare_dram_parameter("wT", [K, E], BF16, isOutput=False)
    bias = nc.declare_dram_parameter("bias", [E], F32, isOutput=False)
    # bf16 output store halves the dominant DMA stream; host upcasts to f32.
    prob = nc.declare_dram_parameter("prob", [bpc, U, N], BF16, isOutput=True)

    with tile.TileContext(nc) as tc, ExitStack() as ctx:
        singles = ctx.enter_context(tc.tile_pool(name="singles", bufs=1))
        pin = ctx.enter_context(tc.tile_pool(name="pin", bufs=4))
        pproj = ctx.enter_context(tc.tile_pool(name="pproj", bufs=3))
        pet = ctx.enter_context(tc.tile_pool(name="pet", bufs=8))
        pprob = ctx.enter_context(tc.tile_pool(name="pprob", bufs=4))
        psmall = ctx.enter_context(tc.tile_pool(name="psmall", bufs=16))
        pps1 = ctx.enter_context(tc.tile_pool(name="pps1", bufs=2, space="PSUM"))
        pps2 = ctx.enter_context(tc.tile_pool(name="pps2", bufs=3, space="PSUM"))

        # ---- resident constants ----
        # wT as 4 k-tiles: wt_sb[p, kt, e] = wT[kt*128+p, e]
        wt_sb = singles.tile([128, 4, E], BF16)
        # scalar (ACT) HWDGE ring: wt/bias stream concurrently with uft0 on
        # the Sync ring instead of ahead of it
        nc.scalar.dma_start(out=wt_sb, in_=wT[:, :].rearrange("(kt p) e -> p kt e", p=128))
        # bias as 2 e-tiles on partitions: b_sb[p, et] = bias[et*128+p]
        b_sb = singles.tile([128, 2], F32)
        nc.scalar.dma_start(out=b_sb, in_=bias[:].rearrange("(et p) -> p et", p=128))

        # ACT exp-table prefetch: the first Exp triggers a ~1.3us
        # ACT_TABLE_LOAD; run a dummy 1-element exp during the load phase so
        # the first real ACTIVATE doesn't pay it on the critical path. Emitted
        # after the wt/bias DMA issues so it doesn't delay them.
        warm = singles.tile([128, 1], F32, name="warm")
        nc.gpsimd.memset(warm, 0.0)
        warm_out = singles.tile([128, 1], F32, name="warm_out")
        nc.scalar.activation(
            out=warm_out, in_=warm, func=mybir.ActivationFunctionType.Exp
        )

        def emit_loads(bi, slot0=False):
            uft = pin.tile([128, 4, U], BF16, tag="uft")
            W = widths[bi]
            eft = pin.tile([128, 2, W], BF16, tag="eft", name=f"eft{bi}")
            if slot0:
                # First slot ramps the pipeline: u-quarter loads so mm1 can
                # start after 256KB, with eft (small) right after the first
                # quarter so mm2/exp of u-tiles 0-1 unblock ~2us sooner.
                usl = slice(0, 256)
                nc.sync.dma_start(
                    out=uft[:, :, usl],
                    in_=ufT[bi, :, usl].rearrange("(kt p) u -> p kt u", p=128),
                )
                nc.sync.dma_start(
                    out=eft, in_=efT[bi, :, :W].rearrange("(et p) n -> p et n", p=128)
                )
                for q in range(1, 4):
                    usl = slice(q * 256, (q + 1) * 256)
                    nc.sync.dma_start(
                        out=uft[:, :, usl],
                        in_=ufT[bi, :, usl].rearrange("(kt p) u -> p kt u", p=128),
                    )
            else:
                nc.sync.dma_start(
                    out=uft, in_=ufT[bi, :, :].rearrange("(kt p) u -> p kt u", p=128)
                )
                nc.sync.dma_start(
                    out=eft, in_=efT[bi, :, :W].rearrange("(et p) n -> p et n", p=128)
                )
            return uft, eft

        def emit_mm1_pair(uft, projT, uc, quarter=False):
            # Both e-half accumulation chains of one u-chunk, interleaved so
            # consecutive MATMULs target different PSUM banks (fill/drain
            # pipelining); each chain's start/stop accumulation unchanged.
            # quarter=True (slot-0 ramp) runs 256-wide so the first chain
            # starts after a quarter of uft has landed.
            fd = 256 if quarter else 512
            usl = slice(uc * fd, (uc + 1) * fd)
            ps = [
                pps1.tile([128, fd], F32, tag="ps1", name=f"ps1_{uc}_{ej}")
                for ej in range(2)
            ]
            for kj in range(4):
                for ej in range(2):
                    nc.tensor.matmul(
                        ps[ej],
                        lhsT=wt_sb[:, kj, ej * 128 : (ej + 1) * 128],
                        rhs=uft[:, kj, usl],
                        start=(kj == 0),
                        stop=(kj == 3),
                    )
            for ej in range(2):
                # relu(x + b) = max(x + b, 0) fused on DVE; casts to bf16
                nc.vector.tensor_scalar(
                    out=projT[:, ej, usl],
                    in0=ps[ej],
                    scalar1=b_sb[:, ej : ej + 1],
                    scalar2=0.0,
                    op0=mybir.AluOpType.add,
                    op1=mybir.AluOpType.max,
                )

        pair_state = {}

        def emit_softmax_tail(bi, ui, ps2):
            # ACT writes a standalone et tile (8 bufs): the critical engine is
            # never gated on store-gang rotation (gang bufs wait on store DMA
            # completion — coupling ACT to DMA round-trips cost ~6us).
            W = widths[bi]
            et = pet.tile([128, W], BF16, tag="et", name=f"et{bi}_{ui}")
            s = psmall.tile([128, 1], F32, tag="s")
            nc.scalar.activation(
                out=et,
                in_=ps2,
                func=mybir.ActivationFunctionType.Exp,
                scale=1.0 / 16.0,
                accum_out=s,
            )
            r = psmall.tile([128, 1], F32, tag="r")
            nc.vector.reciprocal(out=r, in_=s)
            if ui % 4 == 0:
                pair_state["tile"] = pprob.tile(
                    [128, 4, W], BF16, tag="prob", name=f"prob{bi}_{ui}"
                )
            prob_t = pair_state["tile"]
            nc.vector.tensor_scalar_mul(out=prob_t[:, ui % 4, :], in0=et, scalar1=r)
            if ui % 4 == 3:
                base = (ui - 3) * 128
                # SWDGE ring (gpsimd): stores never block loads on the Sync ring
                nc.gpsimd.dma_start(
                    out=prob[bi, base : base + 512, :W].rearrange(
                        "(j p) n -> p j n", p=128
                    ),
                    in_=prob_t,
                )

        def emit_softmax_pair(bi, projT, eft, ui):
            # mm2 for u-tiles ui and ui+1 with chains interleaved across two
            # PSUM tiles (bank alternation), then the two softmax tails.
            W = widths[bi]
            nslices = [slice(0, min(512, W))] + ([slice(512, W)] if W > 512 else [])
            ps2 = [
                pps2.tile([128, W], F32, tag="ps2", name=f"ps2_{bi}_{ui + i}")
                for i in range(2)
            ]
            for ej in range(2):
                for nsl in nslices:
                    for i in range(2):
                        uslice = slice((ui + i) * 128, (ui + i + 1) * 128)
                        nc.tensor.matmul(
                            ps2[i][:, nsl],
                            lhsT=projT[:, ej, uslice],
                            rhs=eft[:, ej, nsl],
                            start=(ej == 0),
                            stop=(ej == 1),
                        )
            for i in range(2):
                emit_softmax_tail(bi, ui + i, ps2[i])

        # Software-pipelined emission: mm1 pairs for batch bi+1 are emitted
        # between softmax pairs of batch bi's second half, so the PE never
        # monopolizes a contiguous window on mm1 while ACT's PSUM backlog
        # drains. Loads run 3 batches ahead on the Sync ring.
        tiles = {0: emit_loads(0, slot0=True)}
        projs = {0: pproj.tile([128, 2, U], BF16, tag="projT", name="projT0")}
        for bi in range(1, min(3, bpc)):
            tiles[bi] = emit_loads(bi)
        # slot-0 ramp: each softmax pair directly follows the mm1 quarter it
        # needs, so the first ACTIVATE unblocks ~5us sooner than emitting all
        # of mm1 up front; batch 1's mm1 halves ride along the tail pairs.
        emit_mm1_pair(tiles[0][0], projs[0], 0, quarter=True)
        for bi in range(bpc):
            uft, eft = tiles[bi]
            projT = projs[bi]
            if bi + 3 < bpc:
                tiles[bi + 3] = emit_loads(bi + 3)
            if bi == 0:
                emit_softmax_pair(bi, projT, eft, 0)
                emit_mm1_pair(uft, projT, 1, quarter=True)
                emit_softmax_pair(bi, projT, eft, 2)
                emit_mm1_pair(uft, projT, 2, quarter=True)
                if bpc > 1:
                    projs[1] = pproj.tile(
                        [128, 2, U], BF16, tag="projT", name="projT1"
                    )
                emit_softmax_pair(bi, projT, eft, 4)
                emit_mm1_pair(uft, projT, 3, quarter=True)
                if bpc > 1:
                    emit_mm1_pair(tiles[1][0], projs[1], 0)
                emit_softmax_pair(bi, projT, eft, 6)
                if bpc > 1:
                    emit_mm1_pair(tiles[1][0], projs[1], 1)
                continue
            emit_softmax_pair(bi, projT, eft, 0)
            if bi + 1 < bpc:
                projs[bi + 1] = pproj.tile(
                    [128, 2, U], BF16, tag="projT", name=f"projT{bi + 1}"
                )
            emit_softmax_pair(bi, projT, eft, 2)
            if bi + 1 < bpc:
                emit_mm1_pair(tiles[bi + 1][0], projs[bi + 1], 0)
            emit_softmax_pair(bi, projT, eft, 4)
            if bi + 1 < bpc:
                emit_mm1_pair(tiles[bi + 1][0], projs[bi + 1], 1)
            emit_softmax_pair(bi, projT, eft, 6)

    # Runs Bacc.compile(): register allocation + event-semaphore splitting.
    nc.finalize()
    return nc


def _prep_inputs(ufeat, efeat, num_enemy, v, g, b):
    """Host-side prep: weight-norm, transpose + bf16 cast, mask bias."""
    ufeat = np.asarray(ufeat, dtype=np.float32)
    efeat = np.asarray(efeat, dtype=np.float32)
    num_enemy = np.asarray(num_enemy).astype(np.int64)
    v = np.asarray(v, dtype=np.float32)
    g = np.float32(np.asarray(g))
    b = np.asarray(b, dtype=np.float32)

    W = (g / np.float32(np.linalg.norm(v))) * v  # [E, K]
    wT = np.ascontiguousarray(W.T).astype(BF16_NP)  # [K, E]

    # [B, K, U] / [B, E, N] bf16 (cast first: halves the transpose traffic)
    ufT = ufeat.astype(BF16_NP).transpose(0, 2, 1)
    efT = np.ascontiguousarray(efeat.astype(BF16_NP).transpose(0, 2, 1))

    # Mask: poison masked efeat columns (n >= num_enemy) with -1e30. Since
    # proj >= 0 (relu) and a proj row is never identically 0 in practice,
    # masked logits land at <= -1e28 and exp underflows to exactly 0 — the
    # same 0 the reference's -1e9 bias produces. num_enemy==0 => all lanes
    # masked => the reference's uniform -1e9 shift cancels in softmax =>
    # leave those batches unpoisoned.
    ne = np.where(num_enemy > 0, num_enemy, N)
    col_masked = np.arange(N)[None, :] >= ne[:, None]  # [B, N]
    efT[np.broadcast_to(col_masked[:, None, :], efT.shape)] = BF16_NP(-1e30)

    return ufT, efT, wT, b


_nc_cache: dict[tuple, bass.Bass] = {}


def run(ufeat, efeat, num_enemy, v, g, b, trace=False):
    ufT, efT, wT, b = _prep_inputs(ufeat, efeat, num_enemy, v, g, b)

    # Masked columns (n >= num_enemy) of the output are exactly 0 and the
    # PJRT path donates zero-initialized output buffers, so the kernel only
    # needs to compute/store columns [0, W) per batch. Sort batches by
    # effective width (descending), assign rank 8k+c to (core c, slot k),
    # and compile the program with a static per-slot width = the slot's max
    # rounded up to 128. Identical widths across cores keeps it SPMD.
    ne = np.asarray(num_enemy).astype(np.int64)
    ne_eff = np.where(ne > 0, ne, N)
    order = np.argsort(-ne_eff, kind="stable")  # descending width ranks
    # Processing order (sigma over the descending-width slots): a narrow
    # slot first (small eft -> exp starts ~8us sooner), the widest while
    # DMA is hot, the narrowest second-to-last (its inefficient 256B-
    # descriptor store drains under the last batch), and a mid slot last
    # (short, efficient final drain).
    sigma = SLOT_SIGMA[:BPC] if BPC == 8 else list(range(BPC))
    slot_ne = ne_eff[order].reshape(BPC, N_CORES)[sigma]
    widths = tuple(
        int(max(128, -(-int(m) // 32) * 32)) for m in slot_ne.max(axis=1)
    )

    key = (BPC, widths)
    if key not in _nc_cache:
        _nc_cache[key] = _build_bass(BPC, widths)
    nc = _nc_cache[key]

    in_maps = []
    perms = []
    for c in range(N_CORES):
        perm = order.reshape(BPC, N_CORES)[sigma][:, c]  # batch per slot
        perms.append(perm)
        in_maps.append({"ufT": ufT[perm], "efT": efT[perm], "wT": wT, "bias": b})

    res = run_bass_kernel_spmd(nc, in_maps, list(range(N_CORES)), trace=trace)
    out = np.empty((B, U, N), dtype=np.float32)
    for c in range(N_CORES):
        o = res.results[c]["prob"].astype(np.float32)
        # Columns [W, N) are masked => exactly 0 by construction. The device
        # never writes them (donated output buffers are zero-filled); zero
        # them here too so correctness never rests on buffer-init behavior.
        for k, w in enumerate(widths):
            o[k, :, w:] = 0.0
        out[perms[c]] = o
    return out, res


def kernel(ufeat, efeat, num_enemy, v, g, b):
    out, _ = run(ufeat, efeat, num_enemy, v, g, b, trace=False)
    return out


# revision 9
# speedup vs baseline: 1.1984x; 1.0239x over previous
"""DotAttackHead kernel for Trainium2 (8 NeuronCores, data-parallel over batch).

prob = softmax(relu(ufeat @ W.T + b) @ efeat.T / sqrt(256) + mask_bias)
W = g * v / ||v||_F

Sharding: batch 64 -> 8 cores x 8 batches (data-parallel). Params replicated.

Host prep: weight-norm W, transpose+bf16-cast of ufeat/efeat (the PE needs
the contraction dim on partitions, and bf16 halves input DMA), and the mask
folded into efeat: masked columns (n >= num_enemy) are set to -1e30, so
masked logits land at <= -1e28 and exp underflows to exactly 0 — the same 0
the reference's -1e9 bias produces.

Device per batch b (software-pipelined across batches):
  mm1:  projT[e,u] = relu(wT.T @ ufT[b] + bias)   (PE bf16; bias+relu fused
        on DVE as tensor_scalar add/max reading PSUM, bf16 out)
  mm2:  psum[u,n]  = projT.T @ efT[b]             (PE bf16, fp32 PSUM)
  soft: e = Exp(psum/16) with accum_out row-sum for free (ACT), r = 1/s
        (DVE reciprocal), prob = e * r (DVE 4x bf16), bf16 DMA out
        (host upcasts to f32).
No max-subtraction: logits are O(+-6) so exp is safe in fp32, and softmax is
shift-invariant, so this matches the reference.

Mask-width specialization: masked output columns are exactly 0, so the
program is compiled (per num_enemy multiset, NEFF-cached) with a static
per-slot column budget: batches sorted by effective width descending,
rank 8k+c -> (core c, slot k), slot width = slot max rounded up to 128.
Only columns [0, W_k) are computed/stored; the rest of each output row is
zeroed (device writes nothing there; host also zeroes defensively).

v2 scheduling (from NTFF profile of the 84-86us baseline):
- Stores issue on the gpsimd SWDGE ring, loads on the Sync HWDGE ring:
  separate DMA rings round-robin at packet granularity, so a store whose
  data isn't ready can no longer head-of-line-block the next batch's loads
  (single-ring FIFO did exactly that), and SDMA drains both streams.
- uft loads are one DMA per batch (2KB descriptors; the old u-half split
  halved descriptor size for no steady-state benefit). Batch 0 keeps the
  split so mm1 starts after 512KB.
- Load lookahead 3 batches (pin bufs=4) keeps ~4MB queued on the load ring.
- PE chains alternate PSUM banks instruction-by-instruction (mm1: the two
  e-half accumulation chains interleaved; mm2: two u-tiles interleaved), so
  consecutive MATMULs hit different banks and fill/drain pipelines instead
  of serializing at the isolated-MM gap.
"""

from contextlib import ExitStack

import ml_dtypes
import numpy as np

import concourse.bass as bass
import concourse.mybir as mybir
import concourse.tile as tile
from concourse import bacc
from concourse.bass_utils import run_bass_kernel_spmd

N_CORES = 8
B = 64
U = 1024  # units
E = 256   # efeat dim
K = 512   # ufeat dim
N = 1024  # enemies
BPC = B // N_CORES  # batches per core
# processing order over descending-width slot ranks (see run())
SLOT_SIGMA = [2, 0, 1, 3, 4, 5, 6, 7]

F32 = mybir.dt.float32
BF16 = mybir.dt.bfloat16
BF16_NP = ml_dtypes.bfloat16

def _build_bass(bpc: int = BPC, widths: tuple = ()) -> bass.Bass:
    if not widths:
        widths = (N,) * bpc
    assert len(widths) == bpc and all(w % 32 == 0 and 128 <= w <= N for w in widths)
    # Bacc (not raw Bass): its finalize() runs generate_event_semaphores,
    # which splits multi-wait instructions to satisfy TRN2's 1-wait limit.
    nc = bacc.Bacc(None, target_bir_lowering=False)

    ufT = nc.declare_dram_parameter("ufT", [bpc, K, U], BF16, isOutput=False)
    efT = nc.declare_dram_parameter("efT", [bpc, E, N], BF16, isOutput=False)
    wT = nc.declare_dram_parameter("wT", [K, E], BF16, isOutput=False)
    bias = nc.declare_dram_parameter("bias", [E], F32, isOutput=False)
    # bf16 output store halves the dominant DMA stream; host upcasts to f32.
    prob = nc.declare_dram_parameter("prob", [bpc, U, N], BF16, isOutput=True)

    with tile.TileContext(nc) as tc, ExitStack() as ctx:
        singles = ctx.enter_context(tc.tile_pool(name="singles", bufs=1))
        pin = ctx.enter_context(tc.tile_pool(name="pin", bufs=4))
        pproj = ctx.enter_context(tc.tile_pool(name="pproj", bufs=3))
        pet = ctx.enter_context(tc.tile_pool(name="pet", bufs=8))
        pprob = ctx.enter_context(tc.tile_pool(name="pprob", bufs=4))
        psmall = ctx.enter_context(tc.tile_pool(name="psmall", bufs=16))
        pps1 = ctx.enter_context(tc.tile_pool(name="pps1", bufs=2, space="PSUM"))
        pps2 = ctx.enter_context(tc.tile_pool(name="pps2", bufs=3, space="PSUM"))

        # ---- resident constants ----
        # wT as 4 k-tiles: wt_sb[p, kt, e] = wT[kt*128+p, e]
        wt_sb = singles.tile([128, 4, E], BF16)
        # scalar (ACT) HWDGE ring: wt/bias stream concurrently with uft0 on
        # the Sync ring instead of ahead of it
        nc.scalar.dma_start(out=wt_sb, in_=wT[:, :].rearrange("(kt p) e -> p kt e", p=128))
        # bias as 2 e-tiles on partitions: b_sb[p, et] = bias[et*128+p]
        b_sb = singles.tile([128, 2], F32)
        nc.scalar.dma_start(out=b_sb, in_=bias[:].rearrange("(et p) -> p et", p=128))

        # ACT exp-table prefetch: the first Exp triggers a ~1.3us
        # ACT_TABLE_LOAD; run a dummy 1-element exp during the load phase so
        # the first real ACTIVATE doesn't pay it on the critical path. Emitted
        # after the wt/bias DMA issues so it doesn't delay them.
        warm = singles.tile([128, 1], F32, name="warm")
        nc.gpsimd.memset(warm, 0.0)
        warm_out = singles.tile([128, 1], F32, name="warm_out")
        nc.scalar.activation(
            out=warm_out, in_=warm, func=mybir.ActivationFunctionType.Exp
        )

        def emit_loads(bi, slot0=False):
            uft = pin.tile([128, 4, U], BF16, tag="uft")
            W = widths[bi]
            eft = pin.tile([128, 2, W], BF16, tag="eft", name=f"eft{bi}")
            if slot0:
                # First slot ramps the pipeline: u-quarter loads so mm1 can
                # start after 256KB, with eft (small) right after the first
                # quarter so mm2/exp of u-tiles 0-1 unblock ~2us sooner.
                usl = slice(0, 256)
                nc.sync.dma_start(
                    out=uft[:, :, usl],
                    in_=ufT[bi, :, usl].rearrange("(kt p) u -> p kt u", p=128),
                )
                nc.sync.dma_start(
                    out=eft, in_=efT[bi, :, :W].rearrange("(et p) n -> p et n", p=128)
                )
                for q in range(1, 4):
                    usl = slice(q * 256, (q + 1) * 256)
                    nc.sync.dma_start(
                        out=uft[:, :, usl],
                        in_=ufT[bi, :, usl].rearrange("(kt p) u -> p kt u", p=128),
                    )
            else:
                nc.sync.dma_start(
                    out=uft, in_=ufT[bi, :, :].rearrange("(kt p) u -> p kt u", p=128)
                )
                nc.sync.dma_start(
                    out=eft, in_=efT[bi, :, :W].rearrange("(et p) n -> p et n", p=128)
                )
            return uft, eft

        def emit_mm1_pair(uft, projT, uc, quarter=False):
            # Both e-half accumulation chains of one u-chunk, interleaved so
            # consecutive MATMULs target different PSUM banks (fill/drain
            # pipelining); each chain's start/stop accumulation unchanged.
            # quarter=True (slot-0 ramp) runs 256-wide so the first chain
            # starts after a quarter of uft has landed.
            fd = 256 if quarter else 512
            usl = slice(uc * fd, (uc + 1) * fd)
            ps = [
                pps1.tile([128, fd], F32, tag="ps1", name=f"ps1_{uc}_{ej}")
                for ej in range(2)
            ]
            for kj in range(4):
                for ej in range(2):
                    nc.tensor.matmul(
                        ps[ej],
                        lhsT=wt_sb[:, kj, ej * 128 : (ej + 1) * 128],
                        rhs=uft[:, kj, usl],
                        start=(kj == 0),
                        stop=(kj == 3),
                    )
            for ej in range(2):
                # relu(x + b) = max(x + b, 0) fused on DVE; casts to bf16
                nc.vector.tensor_scalar(
                    out=projT[:, ej, usl],
                    in0=ps[ej],
                    scalar1=b_sb[:, ej : ej + 1],
                    scalar2=0.0,
                    op0=mybir.AluOpType.add,
                    op1=mybir.AluOpType.max,
                )

        pair_state = {}

        def emit_softmax_tail(bi, ui, ps2):
            # ACT writes a standalone et tile (8 bufs): the critical engine is
            # never gated on store-gang rotation (gang bufs wait on store DMA
            # completion — coupling ACT to DMA round-trips cost ~6us).
            W = widths[bi]
            et = pet.tile([128, W], BF16, tag="et", name=f"et{bi}_{ui}")
            s = psmall.tile([128, 1], F32, tag="s")
            nc.scalar.activation(
                out=et,
                in_=ps2,
                func=mybir.ActivationFunctionType.Exp,
                scale=1.0 / 16.0,
                accum_out=s,
            )
            r = psmall.tile([128, 1], F32, tag="r")
            nc.vector.reciprocal(out=r, in_=s)
            if ui % 4 == 0:
                pair_state["tile"] = pprob.tile(
                    [128, 4, W], BF16, tag="prob", name=f"prob{bi}_{ui}"
                )
            prob_t = pair_state["tile"]
            nc.vector.tensor_scalar_mul(out=prob_t[:, ui % 4, :], in0=et, scalar1=r)
            if ui % 4 == 3:
                base = (ui - 3) * 128
                # SWDGE ring (gpsimd): stores never block loads on the Sync ring
                nc.gpsimd.dma_start(
                    out=prob[bi, base : base + 512, :W].rearrange(
                        "(j p) n -> p j n", p=128
                    ),
                    in_=prob_t,
                )

        def emit_softmax_pair(bi, projT, eft, ui):
            # mm2 for u-tiles ui and ui+1 with chains interleaved across two
            # PSUM tiles (bank alternation), then the two softmax tails.
            W = widths[bi]
            nslices = [slice(0, min(512, W))] + ([slice(512, W)] if W > 512 else [])
            ps2 = [
                pps2.tile([128, W], F32, tag="ps2", name=f"ps2_{bi}_{ui + i}")
                for i in range(2)
            ]
            for ej in range(2):
                for nsl in nslices:
                    for i in range(2):
                        uslice = slice((ui + i) * 128, (ui + i + 1) * 128)
                        nc.tensor.matmul(
                            ps2[i][:, nsl],
                            lhsT=projT[:, ej, uslice],
                            rhs=eft[:, ej, nsl],
                            start=(ej == 0),
                            stop=(ej == 1),
                        )
            for i in range(2):
                emit_softmax_tail(bi, ui + i, ps2[i])

        # Software-pipelined emission: mm1 pairs for batch bi+1 are emitted
        # between softmax pairs of batch bi's second half, so the PE never
        # monopolizes a contiguous window on mm1 while ACT's PSUM backlog
        # drains. Loads run 3 batches ahead on the Sync ring.
        tiles = {0: emit_loads(0, slot0=True)}
        projs = {0: pproj.tile([128, 2, U], BF16, tag="projT", name="projT0")}
        for bi in range(1, min(3, bpc)):
            tiles[bi] = emit_loads(bi)
        # slot-0 ramp: each softmax pair directly follows the mm1 quarter it
        # needs, so the first ACTIVATE unblocks ~5us sooner than emitting all
        # of mm1 up front; batch 1's mm1 halves ride along the tail pairs.
        emit_mm1_pair(tiles[0][0], projs[0], 0, quarter=True)
        for bi in range(bpc):
            uft, eft = tiles[bi]
            projT = projs[bi]
            if bi + 3 < bpc:
                tiles[bi + 3] = emit_loads(bi + 3)
            if bi == 0:
                emit_softmax_pair(bi, projT, eft, 0)
                emit_mm1_pair(uft, projT, 1, quarter=True)
                emit_softmax_pair(bi, projT, eft, 2)
                emit_mm1_pair(uft, projT, 2, quarter=True)
                if bpc > 1:
                    projs[1] = pproj.tile(
                        [128, 2, U], BF16, tag="projT", name="projT1"
                    )
                emit_softmax_pair(bi, projT, eft, 4)
                emit_mm1_pair(uft, projT, 3, quarter=True)
                if bpc > 1:
                    emit_mm1_pair(tiles[1][0], projs[1], 0)
                emit_softmax_pair(bi, projT, eft, 6)
                if bpc > 1:
                    emit_mm1_pair(tiles[1][0], projs[1], 1)
                continue
            emit_softmax_pair(bi, projT, eft, 0)
            if bi + 1 < bpc:
                projs[bi + 1] = pproj.tile(
                    [128, 2, U], BF16, tag="projT", name=f"projT{bi + 1}"
                )
            emit_softmax_pair(bi, projT, eft, 2)
            if bi + 1 < bpc:
                emit_mm1_pair(tiles[bi + 1][0], projs[bi + 1], 0)
            emit_softmax_pair(bi, projT, eft, 4)
            if bi + 1 < bpc:
                emit_mm1_pair(tiles[bi + 1][0], projs[bi + 1], 1)
            emit_softmax_pair(bi, projT, eft, 6)

    # Runs Bacc.compile(): register allocation + event-semaphore splitting.
    nc.finalize()
    return nc


def _prep_inputs(ufeat, efeat, num_enemy, v, g, b):
    """Host-side prep: weight-norm, transpose + bf16 cast, mask bias."""
    ufeat = np.asarray(ufeat, dtype=np.float32)
    efeat = np.asarray(efeat, dtype=np.float32)
    num_enemy = np.asarray(num_enemy).astype(np.int64)
    v = np.asarray(v, dtype=np.float32)
    g = np.float32(np.asarray(g))
    b = np.asarray(b, dtype=np.float32)

    W = (g / np.float32(np.linalg.norm(v))) * v  # [E, K]
    wT = np.ascontiguousarray(W.T).astype(BF16_NP)  # [K, E]

    # [B, K, U] / [B, E, N] bf16 (cast first: halves the transpose traffic)
    ufT = ufeat.astype(BF16_NP).transpose(0, 2, 1)
    efT = np.ascontiguousarray(efeat.astype(BF16_NP).transpose(0, 2, 1))

    # Mask: poison masked efeat columns (n >= num_enemy) with -1e30. Since
    # proj >= 0 (relu) and a proj row is never identically 0 in practice,
    # masked logits land at <= -1e28 and exp underflows to exactly 0 — the
    # same 0 the reference's -1e9 bias produces. num_enemy==0 => all lanes
    # masked => the reference's uniform -1e9 shift cancels in softmax =>
    # leave those batches unpoisoned.
    ne = np.where(num_enemy > 0, num_enemy, N)
    col_masked = np.arange(N)[None, :] >= ne[:, None]  # [B, N]
    efT[np.broadcast_to(col_masked[:, None, :], efT.shape)] = BF16_NP(-1e30)

    return ufT, efT, wT, b


_nc_cache: dict[tuple, bass.Bass] = {}


def run(ufeat, efeat, num_enemy, v, g, b, trace=False):
    ufT, efT, wT, b = _prep_inputs(ufeat, efeat, num_enemy, v, g, b)

    # Masked columns (n >= num_enemy) of the output are exactly 0 and the
    # PJRT path donates zero-initialized output buffers, so the kernel only
    # needs to compute/store columns [0, W) per batch. Sort batches by
    # effective width (descending), assign rank 8k+c to (core c, slot k),
    # and compile the program with a static per-slot width = the slot's max
    # rounded up to 128. Identical widths across cores keeps it SPMD.
    ne = np.asarray(num_enemy).astype(np.int64)
    ne_eff = np.where(ne > 0, ne, N)
    order = np.argsort(-ne_eff, kind="stable")  # descending width ranks
    # Processing order (sigma over the descending-width slots): a narrow
    # slot first (small eft -> exp starts ~8us sooner), the widest while
    # DMA is hot, the narrowest second-to-last (its inefficient 256B-
    # descriptor store drains under the last batch), and a mid slot last
    # (short, efficient final drain).
    sigma = SLOT_SIGMA[:BPC] if BPC == 8 else list(range(BPC))
    slot_ne = ne_eff[order].reshape(BPC, N_CORES)[sigma]
    widths = tuple(
        int(max(128, -(-int(m) // 32) * 32)) for m in slot_ne.max(axis=1)
    )

    key = (BPC, widths)
    if key not in _nc_cache:
        _nc_cache[key] = _build_bass(BPC, widths)
    nc = _nc_cache[key]

    in_maps = []
    perms = []
    for c in range(N_CORES):
        perm = order.reshape(BPC, N_CORES)[sigma][:, c]  # batch per slot
        perms.append(perm)
        in_maps.append({"ufT": ufT[perm], "efT": efT[perm], "wT": wT, "bias": b})

    res = run_bass_kernel_spmd(nc, in_maps, list(range(N_CORES)), trace=trace)
    out = np.empty((B, U, N), dtype=np.float32)
    for c in range(N_CORES):
        o = res.results[c]["prob"].astype(np.float32)
        # Columns [W, N) are masked => exactly 0 by construction. The device
        # never writes them (donated output buffers are zero-filled); zero
        # them here too so correctness never rests on buffer-init behavior.
        for k, w in enumerate(widths):
            o[k, :, w:] = 0.0
        out[perms[c]] = o
    return out, res


def kernel(ufeat, efeat, num_enemy, v, g, b):
    out, _ = run(ufeat, efeat, num_enemy, v, g, b, trace=False)
    return out
